# revision 20
# baseline (speedup 1.0000x reference)
"""2-layer GCN (PyG GCNConv semantics) on 8 Trainium2 NeuronCores.

Strategy (dst-sharded message passing; v2):
  - Nodes are split into 8 contiguous blocks of 6250 rows; core c owns output
    rows [6250c, 6250(c+1)).  Real edges (no self-loops) are partitioned by
    destination core, then by 256-node destination windows, then packed into
    128-edge tiles.  Per edge tile: dma_gather x[src] rows (one 512B
    descriptor per edge; HBM random-row fetch at ~6.3 ns/row is the
    bottleneck), build a norm-scaled one-hot dst selection matrix in one DVE
    tensor_scalar, and accumulate aggT[feat, dst] += msg^T @ sel on the PE in
    float32r.
  - Self-loops skip the gather: per 128-row half-window, a contiguous HWDGE
    load of x rows + per-partition dinv^2 scale + PE transpose-accumulate
    (matmul against a 128x128 identity) adds diag(dinv^2) x directly into the
    aggregation PSUM.
  - Window flush (layer 1): h1T = relu(W1^T @ aggT + b1); pT = W2^T @ h1T;
    PE-transpose to row-major and store p = h1 @ W2 rows to p_mine.
  - AllGather is CHUNKED and overlapped with layer 1: p rows are renumbered
    chunk-major ([chunk][core][local]) and each chunk's AllGather (Shared
    addr-space output) is issued two windows after its producer windows
    finish, so the collective rides under the layer-1 gathers.
  - Layer 2 re-uses the same edge structure with renumbered src rows, and
    gathers 512B per descriptor (elem_size=128, elem_step=64: each descriptor
    reads p[src] plus the following row as overshoot) because sub-512B
    descriptors are measurably slower (~8.4 vs ~6.3 ns/row).  Self-loops add
    diag(dinv^2) p from the local p_mine rows, no collective dependency.
  - dma_gather indices are int16, so row tables are addressed via two calls
    per window: "lo" (row < 32768) and "hi" (an offset view).  Padding uses
    trailing -1 indices which the DGE skips; padded lanes are killed in the
    selection matrix (dst_local = -1 never matches the iota).

Host-side work is index preprocessing only (degrees/norms from edge_index,
sorting, packing); all FLOPs on the gathered/aggregated features run on
device.
"""

import contextlib
import os
import sys

import numpy as np

for _p in ("/opt/trn_rl_repo", "/root/.axon_site/_ro/trn_rl_repo"):
    if os.path.isdir(_p) and _p not in sys.path:
        sys.path.insert(0, _p)

import concourse.bacc as bacc
import concourse.tile as tile
from concourse import mybir
from concourse.bass_utils import run_bass_kernel_spmd
import bass_rust

P = 128
N_NODES = 50000
C_IN = 128
C_HID = 128
C_OUT = 64
CORES = 8
BLOCK = N_NODES // CORES          # 6250
WIN = 256                         # dst nodes per PSUM window
NW = -(-BLOCK // WIN)             # 25 windows per core (last has 106 rows)
SPLIT = 32768                     # int16-addressable base-table rows
GAT_BUFS = 4                      # msg tile buffering

# AllGather chunks: windows per chunk (sum = NW).  Sized so late chunks are
# small (little exposed collective after layer 1 ends).
CHUNK_WINS = [7, 7, 7, 3, 1]
CHUNK_ROWS = []                   # p-rows per core in each chunk
_acc = 0
for _cw in CHUNK_WINS:
    _lo = _acc * WIN
    _acc += _cw
    CHUNK_ROWS.append(min(_acc * WIN, BLOCK) - _lo)
CHUNK_STARTS = np.concatenate([[0], np.cumsum([CORES * r for r in CHUNK_ROWS])])
# collective for chunk k is issued after this layer-1 window index:
CHUNK_ISSUE_AFTER = []
_acc = 0
for _cw in CHUNK_WINS:
    _acc += _cw
    CHUNK_ISSUE_AFTER.append(min(_acc - 1 + 2, NW - 1))

DEBUG_STAGE = 0                   # 0 = full; 1 = layer 1 only (no AG/L2)


def _renumber():
    """node id -> chunk-major row in p_full: [chunk][core][local-in-chunk]."""
    n = np.arange(N_NODES, dtype=np.int64)
    c = n // BLOCK
    l = n % BLOCK
    w = l // WIN
    # chunk id per window
    w2k = np.zeros(NW, np.int64)
    _acc = 0
    for k, cw in enumerate(CHUNK_WINS):
        w2k[_acc:_acc + cw] = k
        _acc += cw
    k = w2k[w]
    chunk_lo = np.concatenate([[0], np.cumsum(CHUNK_WINS)])[:-1] * WIN  # local row at chunk start
    sizes = np.asarray(CHUNK_ROWS, dtype=np.int64)
    return CHUNK_STARTS[k] + c * sizes[k] + (l - chunk_lo[k])


def _pack(rows, core, win, dloc, nrm):
    """Partition edges by (core, window, lo/hi), pad tiles, build arrays."""
    is_hi = (rows >= SPLIT).astype(np.int64)
    key = (core * NW + win) * 2 + is_hi
    order = np.argsort(key, kind="stable")
    cnt = np.bincount(key, minlength=CORES * NW * 2)
    cnt3 = cnt.reshape(CORES, NW, 2)
    t_lo = int(-(-cnt3[:, :, 0].max() // P))
    t_hi = int(-(-cnt3[:, :, 1].max() // P))
    s_row = rows[order]
    s_nrm = nrm[order]
    s_dloc = dloc[order]
    bounds = np.concatenate([[0], np.cumsum(cnt)])

    t_w = t_lo + t_hi
    per_core = []
    for c in range(CORES):
        idx_lo = np.full((NW, t_lo * P), -1, np.int16)
        idx_hi = np.full((NW, t_hi * P), -1, np.int16)
        dl = np.full((NW, t_w * P), -1.0, np.float32)
        nm = np.zeros((NW, t_w * P), np.float32)
        cnts = np.zeros((NW, 2), np.int32)
        for w in range(NW):
            k = (c * NW + w) * 2
            lo0, lo1 = bounds[k], bounds[k + 1]
            hi0, hi1 = bounds[k + 1], bounds[k + 2]
            nlo, nhi = lo1 - lo0, hi1 - hi0
            idx_lo[w, :nlo] = s_row[lo0:lo1].astype(np.int16)
            idx_hi[w, :nhi] = (s_row[hi0:hi1] - SPLIT).astype(np.int16)
            dl[w, :nlo] = s_dloc[lo0:lo1]
            dl[w, t_lo * P:t_lo * P + nhi] = s_dloc[hi0:hi1]
            nm[w, :nlo] = s_nrm[lo0:lo1]
            nm[w, t_lo * P:t_lo * P + nhi] = s_nrm[hi0:hi1]
            cnts[w, 0] = nlo
            cnts[w, 1] = nhi

        # int16 index arrays: idx i of a call lives at [i % 16, i // 16],
        # replicated across the 8 groups of 16 partitions (one per Q7 core).
        def wrap16(a):  # [NW, L] -> [128, NW * L / 16]
            L = a.shape[1]
            w16 = a.reshape(NW, L // 16, 16).transpose(2, 0, 1).reshape(16, -1)
            return np.tile(w16, (8, 1))

        def lanes(a):  # [NW, t_w * P] -> [128, NW * t_w]
            return a.reshape(NW, t_w, P).transpose(2, 0, 1).reshape(P, -1)

        per_core.append({
            "idx_lo": wrap16(idx_lo) if t_lo else np.zeros((P, 0), np.int16),
            "idx_hi": wrap16(idx_hi) if t_hi else np.zeros((P, 0), np.int16),
            "dloc": lanes(dl),
            "nrm": lanes(nm),
            "cnt": cnts.reshape(1, -1),
        })
    # first tile that may contain an unwritten (padded) slot, per segment
    m_lo = int(cnt3[:, :, 0].min()) // P if t_lo else 0
    m_hi = int(cnt3[:, :, 1].min()) // P if t_hi else 0
    return t_lo, t_hi, per_core, m_lo, m_hi


def _preprocess(edge_index):
    src = np.asarray(edge_index[0], dtype=np.int64)
    dst = np.asarray(edge_index[1], dtype=np.int64)

    deg = np.bincount(dst, minlength=N_NODES).astype(np.float64) + 1.0
    dinv = 1.0 / np.sqrt(deg)
    nrm = (dinv[src] * dinv[dst]).astype(np.float32)

    core = dst // BLOCK
    win = (dst % BLOCK) // WIN
    dloc = ((dst % BLOCK) % WIN).astype(np.float32)

    # Layer-1 x copies are rotated per core (own block first) so the static
    # self-loop row addresses hit the core's own rows; gather rows rotate too.
    r1 = (src - core * BLOCK) % N_NODES
    t1 = _pack(r1, core, win, dloc, nrm)
    r2 = _renumber()[src]
    t2 = _pack(r2, core, win, dloc, nrm)

    # self-loop scales: per core, [NW*2] columns of 128 per-partition values
    dinv2 = (dinv * dinv).astype(np.float32)
    sc = np.zeros((CORES, NW * 2, P), np.float32)
    for c in range(CORES):
        for w in range(NW):
            for h in range(2):
                r0 = w * WIN + h * P
                rh = min(P, BLOCK - r0)
                if rh > 0:
                    sc[c, 2 * w + h, :rh] = dinv2[c * BLOCK + r0:c * BLOCK + r0 + rh]
    return t1, t2, sc


_BUILD_CACHE = {}


def _build(t1_lo, t1_hi, t2_lo, t2_hi, m_lo=0, m_hi=0):
    key = (t1_lo, t1_hi, t2_lo, t2_hi, m_lo, m_hi, DEBUG_STAGE, GAT_BUFS)
    if key in _BUILD_CACHE:
        return _BUILD_CACHE[key]

    t1_w = t1_lo + t1_hi
    t2_w = t2_lo + t2_hi
    tot1 = NW * t1_w
    tot2 = NW * t2_w
    # meta f32 columns:
    # [dloc1 | nrm1 | dloc2 | nrm2 | iota(WIN) | b1 | b2 | sc1(NW*2) | sc2(NW*2)]
    c_d1, c_n1 = 0, tot1
    c_d2, c_n2 = 2 * tot1, 2 * tot1 + tot2
    c_iota = 2 * tot1 + 2 * tot2
    c_b1 = c_iota + WIN
    c_b2 = c_b1 + 1
    c_sc1 = c_b2 + 1
    c_sc2 = c_sc1 + NW * 2
    meta_cols = c_sc2 + NW * 2

    f32, f32r, i16 = mybir.dt.float32, mybir.dt.float32r, mybir.dt.int16
    RELU = mybir.ActivationFunctionType.Relu
    COPY = mybir.ActivationFunctionType.Copy
    IDENT = mybir.ActivationFunctionType.Identity

    nc = bacc.Bacc("TRN2", num_devices=CORES, num_swdge_queues=4)
    x_ext = nc.dram_tensor("x", [N_NODES, C_IN], f32r, kind="ExternalInput")
    il1_ext = nc.dram_tensor("il1", [P, NW * t1_lo * 8], i16, kind="ExternalInput")
    ih1_ext = nc.dram_tensor("ih1", [P, NW * t1_hi * 8], i16, kind="ExternalInput")
    il2_ext = nc.dram_tensor("il2", [P, NW * t2_lo * 8], i16, kind="ExternalInput")
    ih2_ext = nc.dram_tensor("ih2", [P, NW * t2_hi * 8], i16, kind="ExternalInput")
    meta_ext = nc.dram_tensor("meta", [P, meta_cols], f32, kind="ExternalInput")
    cnt_ext = nc.dram_tensor("cnt", [1, NW * 4], mybir.dt.int32, kind="ExternalInput")
    w_ext = nc.dram_tensor("wts", [P, 384], f32r, kind="ExternalInput")
    out_ext = nc.dram_tensor("out", [BLOCK, C_OUT], f32, kind="ExternalOutput")

    with tile.TileContext(nc) as tc:
        with tc.tile_pool(name="const", bufs=1) as cpool, \
             tc.tile_pool(name="gat", bufs=GAT_BUFS) as gpool, \
             tc.tile_pool(name="selfw", bufs=4) as spool, \
             tc.tile_pool(name="work", bufs=3) as wpool, \
             tc.tile_pool(name="flush", bufs=2) as fpool, \
             tc.tile_pool(name="dram", bufs=1, space="DRAM") as dpool, \
             tc.tile_pool(name="ps_agg", bufs=2, space="PSUM") as ps_agg, \
             tc.tile_pool(name="ps_z", bufs=2, space="PSUM") as ps_z, \
             tc.tile_pool(name="ps_pt", bufs=2, space="PSUM") as ps_pt, \
             tc.tile_pool(name="ps_rm", bufs=1, space="PSUM") as ps_rm, \
             tc.tile_pool(name="ps_io", bufs=1, space="PSUM") as ps_io:

            il1_s = cpool.tile([P, NW * t1_lo * 8], i16)
            ih1_s = cpool.tile([P, NW * t1_hi * 8], i16)
            il2_s = cpool.tile([P, NW * t2_lo * 8], i16)
            ih2_s = cpool.tile([P, NW * t2_hi * 8], i16)
            meta_s = cpool.tile([P, meta_cols], f32)
            w_s = cpool.tile([P, 384], f32r)
            cnt_s = cpool.tile([1, NW * 4], mybir.dt.int32)
            nc.sync.dma_start(out=cnt_s[:], in_=cnt_ext[:])
            nc.sync.dma_start(out=il1_s[:], in_=il1_ext[:])
            nc.sync.dma_start(out=ih1_s[:], in_=ih1_ext[:])
            nc.sync.dma_start(out=il2_s[:], in_=il2_ext[:])
            nc.sync.dma_start(out=ih2_s[:], in_=ih2_ext[:])
            nc.sync.dma_start(out=meta_s[:], in_=meta_ext[:])
            nc.sync.dma_start(out=w_s[:], in_=w_ext[:])

            p_mine = dpool.tile([BLOCK + P, C_OUT], f32r)
            p_full = dpool.tile([N_NODES + 1, C_OUT], f32r)

            # iota staged in PSUM: the per-tile DVE sel-build then reads its
            # big operand via the PSUM port instead of SBUF, keeping the
            # DVE/GpSimd shared SBUF port pair free for SWDGE descriptor
            # writes (else the gathers starve behind DVE 2-port ops).
            iota_ps = ps_io.tile([P, WIN], f32, space="PSUM")
            nc.scalar.copy(out=iota_ps[:], in_=meta_s[:, c_iota:c_iota + WIN])

            # zero the pad rows read by self-loop loads / overshoot gathers
            zpad = cpool.tile([P, C_OUT], f32r)
            nc.vector.memset(zpad[:].bitcast(f32), 0.0)
            nc.sync.dma_start(out=p_mine[BLOCK:BLOCK + P, :], in_=zpad[:])
            if DEBUG_STAGE != 1:
                nc.sync.dma_start(out=p_full[N_NODES:N_NODES + 1, :], in_=zpad[0:1, :])

            # overshoot views of p_full: 512B reads with 256B row step
            pf_lo = p_full[:].copy()
            pf_lo.ap = bass_rust.VecI64Pair([[C_OUT, N_NODES], [1, 2 * C_OUT]])
            pf_hi = p_full[SPLIT:, :].copy()
            pf_hi.ap = bass_rust.VecI64Pair(
                [[C_OUT, N_NODES - SPLIT], [1, 2 * C_OUT]])

            _rctx = contextlib.ExitStack()
            rlo = _rctx.enter_context(nc.gpsimd.register("rlo"))
            rhi = _rctx.enter_context(nc.gpsimd.register("rhi"))

            def window_tiles(w, msg, t_w, feat, fwid, c_d, c_n, self_fn):
                """Edge-tile accumulation into one PSUM window + self-loops.

                msg tile is [P, t_w, fwid]; matmuls read [:, t, :feat].
                self_fn(agg, h) adds the self-loop contribution for
                half-window h (called between tile 0 and tile 1).
                """
                agg = ps_agg.tile([P, WIN], f32, space="PSUM", tag="agg")
                halves = -(-min(WIN, BLOCK - w * WIN) // P)
                for t in range(t_w):
                    col = w * t_w + t
                    sel = wpool.tile([P, WIN], f32r, tag="sel")
                    nc.vector.tensor_scalar(
                        out=sel[:],
                        in0=iota_ps[:],
                        scalar1=meta_s[:, c_d + col:c_d + col + 1],
                        scalar2=meta_s[:, c_n + col:c_n + col + 1],
                        op0=mybir.AluOpType.is_equal,
                        op1=mybir.AluOpType.mult,
                    )
                    nc.tensor.matmul(
                        out=agg[:feat, :], lhsT=msg[:, t, 0:feat], rhs=sel[:],
                        start=(t == 0), stop=(t == t_w - 1),
                    )
                    if t == 0:
                        for h in range(halves):
                            self_fn(agg, h)
                return agg

            def store_rowmajor(w, colT_s, dram_dst, dt_out):
                rows = min(WIN, BLOCK - w * WIN)
                for h in range((rows + P - 1) // P):
                    rh = min(P, rows - h * P)
                    rm = ps_rm.tile([P, C_OUT], f32r, space="PSUM", tag="rm")
                    nc.tensor.transpose(
                        out=rm[:],
                        in_=colT_s[:, h * P:(h + 1) * P],
                        identity=w_s[0:C_OUT, 192:256],
                    )
                    rm_s = fpool.tile([P, C_OUT], dt_out, tag="rm_s")
                    nc.scalar.activation(out=rm_s[:], in_=rm[:].bitcast(f32), func=COPY)
                    r0 = w * WIN + h * P
                    nc.sync.dma_start(out=dram_dst[r0:r0 + rh, :], in_=rm_s[:rh, :])

            # ---------------- layer 1 ----------------
            for w in range(NW):
                msg = gpool.tile([P, max(t1_w, t2_w), C_IN], f32r, tag="msg")
                if w < GAT_BUFS:
                    # only the pad slots (tiles past the min valid count) can
                    # stay unwritten by every gather; zero just those
                    if m_lo < t1_lo:
                        nc.vector.memset(
                            msg[:, m_lo:t1_lo, :].rearrange("p c e -> p (c e)").bitcast(f32), 0.0)
                    if m_hi < max(t1_w, t2_w) - t1_lo:
                        nc.vector.memset(
                            msg[:, t1_lo + m_hi:max(t1_w, t2_w), :].rearrange("p c e -> p (c e)").bitcast(f32), 0.0)
                nc.gpsimd.reg_load(rlo, cnt_s[0:1, 2 * w:2 * w + 1])
                nc.gpsimd.reg_load(rhi, cnt_s[0:1, 2 * w + 1:2 * w + 2])
                if t1_lo:
                    nc.gpsimd.dma_gather(
                        out_ap=msg[:, 0:t1_lo, :], in_ap=x_ext[:],
                        idxs_ap=il1_s[:, w * t1_lo * 8:(w + 1) * t1_lo * 8],
                        num_idxs=t1_lo * P, num_idxs_reg=rlo, elem_size=C_IN,
                        single_packet=False, queue_num=(w % 2) * 2,
                    )
                if t1_hi:
                    nc.gpsimd.dma_gather(
                        out_ap=msg[:, t1_lo:t1_w, :], in_ap=x_ext[SPLIT:, :],
                        idxs_ap=ih1_s[:, w * t1_hi * 8:(w + 1) * t1_hi * 8],
                        num_idxs=t1_hi * P, num_idxs_reg=rhi, elem_size=C_IN,
                        single_packet=False, queue_num=(w % 2) * 2 + 1,
                    )

                def self1(agg, h, w=w):
                    xw = spool.tile([P, C_IN], f32r, tag="xw")
                    r0 = w * WIN + h * P
                    nc.sync.dma_start(out=xw[:], in_=x_ext[r0:r0 + P, :])
                    # device-side row offset: each core reads its own block
                    xs = spool.tile([P, C_IN], f32r, tag="xs")
                    nc.scalar.activation(
                        out=xs[:], in_=xw[:].bitcast(f32), func=COPY,
                        scale=meta_s[:, c_sc1 + 2 * w + h:c_sc1 + 2 * w + h + 1])
                    nc.tensor.matmul(
                        out=agg[:, h * P:(h + 1) * P], lhsT=xs[:],
                        rhs=w_s[:, 256:384], start=False, stop=False)

                agg = window_tiles(w, msg, t1_w, C_IN, C_IN, c_d1, c_n1, self1)

                agg_s = wpool.tile([P, WIN], f32r, tag="agg_s")
                nc.scalar.activation(out=agg_s[:], in_=agg[:], func=COPY)
                z = ps_z.tile([P, WIN], f32, space="PSUM", tag="z")
                nc.tensor.matmul(out=z[:], lhsT=w_s[:, 0:C_HID], rhs=agg_s[:],
                                 start=True, stop=True)
                h1_s = wpool.tile([P, WIN], f32r, tag="h1")
                nc.scalar.activation(out=h1_s[:], in_=z[:], func=RELU,
                                     bias=meta_s[:, c_b1:c_b1 + 1])
                pt = ps_pt.tile([C_OUT, WIN], f32, space="PSUM", tag="pt")
                nc.tensor.matmul(out=pt[:], lhsT=w_s[:, 128:128 + C_OUT],
                                 rhs=h1_s[:], start=True, stop=True)
                pt_s = fpool.tile([C_OUT, WIN], f32r, tag="pt_s")
                nc.scalar.activation(out=pt_s[:], in_=pt[:], func=COPY)
                store_rowmajor(w, pt_s, out_ext if DEBUG_STAGE == 1 else p_mine,
                               f32 if DEBUG_STAGE == 1 else f32r)

                # chunked AllGather, issued lagging its producer windows
                if DEBUG_STAGE != 1:
                    for k, after in enumerate(CHUNK_ISSUE_AFTER):
                        if after == w:
                            r0 = int(np.cumsum([0] + CHUNK_ROWS)[k])
                            rows = CHUNK_ROWS[k]
                            g0 = int(CHUNK_STARTS[k])
                            nc.gpsimd.collective_compute(
                                "AllGather", mybir.AluOpType.bypass,
                                replica_groups=[list(range(CORES))],
                                ins=[p_mine[r0:r0 + rows, :]],
                                outs=[p_full[g0:g0 + CORES * rows, :]],
                            )

            # ---------------- layer 2 ----------------
            for w in range(NW if DEBUG_STAGE != 1 else 0):
                msg = gpool.tile([P, max(t1_w, t2_w), C_IN], f32r, tag="msg")
                nc.gpsimd.reg_load(rlo, cnt_s[0:1, 2 * NW + 2 * w:2 * NW + 2 * w + 1])
                nc.gpsimd.reg_load(rhi, cnt_s[0:1, 2 * NW + 2 * w + 1:2 * NW + 2 * w + 2])
                if t2_lo:
                    nc.gpsimd.dma_gather(
                        out_ap=msg[:, 0:t2_lo, :], in_ap=pf_lo,
                        idxs_ap=il2_s[:, w * t2_lo * 8:(w + 1) * t2_lo * 8],
                        num_idxs=t2_lo * P, num_idxs_reg=rlo, elem_size=2 * C_OUT,
                        elem_step=C_OUT,
                        single_packet=False, queue_num=(w % 2) * 2,
                    )
                if t2_hi:
                    nc.gpsimd.dma_gather(
                        out_ap=msg[:, t2_lo:t2_w, :], in_ap=pf_hi,
                        idxs_ap=ih2_s[:, w * t2_hi * 8:(w + 1) * t2_hi * 8],
                        num_idxs=t2_hi * P, num_idxs_reg=rhi, elem_size=2 * C_OUT,
                        elem_step=C_OUT,
                        single_packet=False, queue_num=(w % 2) * 2 + 1,
                    )

                def self2(agg, h, w=w):
                    pw = spool.tile([P, C_OUT], f32r, tag="pw")
                    r0 = w * WIN + h * P
                    nc.sync.dma_start(out=pw[:], in_=p_mine[r0:r0 + P, :])
                    ps = spool.tile([P, C_OUT], f32r, tag="ps")
                    nc.scalar.activation(
                        out=ps[:], in_=pw[:].bitcast(f32), func=COPY,
                        scale=meta_s[:, c_sc2 + 2 * w + h:c_sc2 + 2 * w + h + 1])
                    nc.tensor.matmul(
                        out=agg[:C_OUT, h * P:(h + 1) * P], lhsT=ps[:],
                        rhs=w_s[:, 256:384], start=False, stop=False)

                agg = window_tiles(w, msg, t2_w, C_OUT, C_IN, c_d2, c_n2, self2)

                o_s = fpool.tile([C_OUT, WIN], f32r, tag="o_s")
                nc.scalar.activation(out=o_s[:], in_=agg[:C_OUT, :], func=IDENT,
                                     bias=meta_s[0:C_OUT, c_b2:c_b2 + 1])
                store_rowmajor(w, o_s, out_ext, f32)

            _rctx.close()

    nc.compile()
    layout = dict(c_d1=c_d1, c_n1=c_n1, c_d2=c_d2, c_n2=c_n2, c_iota=c_iota,
                  c_b1=c_b1, c_b2=c_b2, c_sc1=c_sc1, c_sc2=c_sc2,
                  meta_cols=meta_cols)
    _BUILD_CACHE[key] = (nc, layout)
    return nc, layout


def _make_inputs(x, W1, b1, W2, b2, t1, t2, sc, layout):
    t1_lo, t1_hi, pc1 = t1[:3]
    t2_lo, t2_hi, pc2 = t2[:3]
    tot1 = NW * (t1_lo + t1_hi)
    tot2 = NW * (t2_lo + t2_hi)
    L = layout

    wts = np.zeros((P, 384), np.float32)
    wts[:, 0:128] = W1
    wts[:128, 128:192] = W2
    wts[0:64, 192:256] = np.eye(64, dtype=np.float32)
    wts[:, 256:384] = np.eye(128, dtype=np.float32)

    x = np.ascontiguousarray(x, dtype=np.float32)
    in_maps = []
    for c in range(CORES):
        meta = np.zeros((P, L["meta_cols"]), np.float32)
        meta[:, L["c_d1"]:L["c_d1"] + tot1] = pc1[c]["dloc"]
        meta[:, L["c_n1"]:L["c_n1"] + tot1] = pc1[c]["nrm"]
        meta[:, L["c_d2"]:L["c_d2"] + tot2] = pc2[c]["dloc"]
        meta[:, L["c_n2"]:L["c_n2"] + tot2] = pc2[c]["nrm"]
        meta[:, L["c_iota"]:L["c_iota"] + WIN] = np.arange(WIN, dtype=np.float32)[None, :]
        meta[:, L["c_b1"]] = b1
        meta[:C_OUT, L["c_b2"]] = b2
        meta[:, L["c_sc1"]:L["c_sc1"] + NW * 2] = sc[c].T
        meta[:, L["c_sc2"]:L["c_sc2"] + NW * 2] = sc[c].T
        cnt = np.concatenate([pc1[c]["cnt"], pc2[c]["cnt"]], axis=1)

        # rotate x so this core's block is first (static self-loop addresses)
        xc = np.concatenate([x[c * BLOCK:], x[:c * BLOCK]], axis=0)
        in_maps.append({
            "x": xc,
            "il1": pc1[c]["idx_lo"],
            "ih1": pc1[c]["idx_hi"],
            "il2": pc2[c]["idx_lo"],
            "ih2": pc2[c]["idx_hi"],
            "meta": meta,
            "cnt": cnt,
            "wts": wts,
        })
    return in_maps


def kernel(x, edge_index, W1, b1, W2, b2):
    x = np.asarray(x, dtype=np.float32)
    W1 = np.asarray(W1, dtype=np.float32)
    b1 = np.asarray(b1, dtype=np.float32)
    W2 = np.asarray(W2, dtype=np.float32)
    b2 = np.asarray(b2, dtype=np.float32)

    t1, t2, sc = _preprocess(np.asarray(edge_index))
    if t1[0] == t2[0] and t1[1] == t2[1]:
        m_lo, m_hi = min(t1[3], t2[3]), min(t1[4], t2[4])
    else:
        m_lo, m_hi = 0, 0
    nc, layout = _build(t1[0], t1[1], t2[0], t2[1], m_lo, m_hi)
    in_maps = _make_inputs(x, W1, b1, W2, b2, t1, t2, sc, layout)
    res = run_bass_kernel_spmd(nc, in_maps, list(range(CORES)))
    out = np.concatenate([res.results[c]["out"] for c in range(CORES)], axis=0)
    return out.astype(np.float32)
